# revision 15
# baseline (speedup 1.0000x reference)
"""Trainium2 Bass kernel for AutoRegressiveLSTMEncoder.

Strategy: pure data parallel over 8 NeuronCores (batch 32768 -> 4096/core).
All tensors live feature-on-partition / batch-on-free ("transposed") so every
matmul is lhsT.T @ rhs with K on partitions.

Key algebraic optimizations:
  - softmax(log(softplus(s)+eps)) == (softplus(s)+eps) / sum(softplus(s)+eps)
    -> no exp/log needed, and no max-subtraction (values are bounded).
  - The input-side term W_ih[:, :H] @ t_h + b_ih + b_hh is step-invariant:
    precompute once as G0 (saves 1/3 of the per-step FLOPs).
  - Per-step gates = G0 + W_ih[:, H:] @ e + W_hh @ h  (bf16 matmuls, fp32 acc).

Recurrent state (h as 64 per-(k,chunk) tiles, p per-chunk) lives in SBUF for
all 32 steps -- no DRAM round-trip on the latency-critical recurrence. New h
is computed into temps and committed to the resident tiles only after the
z-phase matmuls consume them (the in-chunk WAR is chunk-local). Only the
step-invariant G0 (read-only) and the cell state c (1-step slack) stream
through DRAM, plus the p_all output.

The 32 LSTM steps run in a For_i hardware loop (8 iterations x 4 steps for
static ping-pong c addressing and fewer loop-boundary syncs); per-step probs
are written phase-major (p_all[4][8][A][B_local]) so the only dynamic address
is the loop counter itself. Host reassembles [B, D, A].
"""

import sys

sys.path.insert(0, "/opt/trn_rl_repo")

import numpy as np
import ml_dtypes
from contextlib import ExitStack

import concourse.bass as bass
import concourse.bacc as bacc
import concourse.tile as tile
from concourse import mybir

AF = mybir.ActivationFunctionType
DT = mybir.dt

# Problem dims (hardcoded per contest contract)
B, E, D, A, H = 32768, 300, 32, 64, 1024
G4 = 4 * H  # 4096
NCORES = 8
BL = B // NCORES  # 4096
NT = 512  # moving free-dim per matmul (one fp32 PSUM bank)
EPS = 1e-6
KXP = 384  # E=300 padded to 3*128


UNROLL = 4


def build_nc(BL=BL, NB=None, nsteps=D, use_for_i=True):
    """Build the SPMD Bass program for one core handling BL batch elements."""
    if NB is None:
        NB = BL // NT
    assert BL == NB * NT and nsteps % UNROLL == 0
    NS2 = nsteps // UNROLL

    nc = bacc.Bacc("TRN2", target_bir_lowering=False, debug=False)
    f32, bf = DT.float32, DT.bfloat16

    # ---- external inputs (host pre-tiled / pre-transposed / pre-cast) ----
    xT = nc.dram_tensor("xT", (3, 128, BL), bf, kind="ExternalInput")
    WxhT = nc.dram_tensor("WxhT", (3, 128, H), bf, kind="ExternalInput")
    bxh = nc.dram_tensor("bxh", (128, 8), f32, kind="ExternalInput")
    WihAT = nc.dram_tensor("WihAT", (8, 128, G4), bf, kind="ExternalInput")
    WbigT = nc.dram_tensor("WbigT", (A, G4), bf, kind="ExternalInput")
    WhhT = nc.dram_tensor("WhhT", (8, 128, G4), bf, kind="ExternalInput")
    bg = nc.dram_tensor("bg", (128, 32), f32, kind="ExternalInput")
    WhzT = nc.dram_tensor("WhzT", (8, 128, A), bf, kind="ExternalInput")
    bhz = nc.dram_tensor("bhz", (A, 1), f32, kind="ExternalInput")
    onesA = nc.dram_tensor("onesA", (A, 1), f32, kind="ExternalInput")
    ones1 = nc.dram_tensor("ones1", (1, 128), f32, kind="ExternalInput")

    # ---- output: parity-major probs ----
    p_all = nc.dram_tensor("p_all", (UNROLL, NS2, A, BL), f32, kind="ExternalOutput")

    # ---- internal DRAM scratch ----
    th_d = nc.dram_tensor("th_d", (8, 128, BL), bf, kind="Internal")
    G0_d = nc.dram_tensor("G0_d", (NB, 8, 128, 4 * NT), bf, kind="Internal")
    c_d = [
        nc.dram_tensor(f"c_d{i}", (NB, 8, 128, NT), f32, kind="Internal")
        for i in (0, 1)
    ]

    with tile.TileContext(nc) as tc, ExitStack() as ctx:
        # ---- SBUF-resident recurrent state (h, p stay on-chip all 32 steps;
        # per-(k, n) tiles so WAR tracking is chunk-local) ----
        hres_pool = ctx.enter_context(tc.tile_pool(name="hres", bufs=1))
        hres = [
            [hres_pool.tile([128, NT], bf, tag=f"h{k}_{n}", name=f"h{k}_{n}") for n in range(NB)]
            for k in range(8)
        ]
        pres = [hres_pool.tile([A, NT], bf, tag=f"p{n}", name=f"p{n}") for n in range(NB)]

        # ================= prologue =================
        with ExitStack() as pro:
            cpool = pro.enter_context(tc.tile_pool(name="pc", bufs=1))
            pspool = pro.enter_context(tc.tile_pool(name="pps", bufs=8, space="PSUM"))

            # zero-init state buffers (set 0)
            ztile = cpool.tile([128, NT], f32, tag="z32")
            nc.vector.memset(ztile[:], 0.0)
            for r in range(8):
                for n in range(NB):
                    nc.sync.dma_start(c_d[0][n, r], ztile[:])
                    nc.vector.memset(hres[r][n][:], 0.0)
            for n in range(NB):
                nc.vector.memset(pres[n][:], 0.0)

            # t_h = tanh(W_xh @ xT + b_xh)
            wxh = [cpool.tile([128, H], bf, tag=f"wxh{k}", name=f"wxh{k}") for k in range(3)]
            for k in range(3):
                nc.sync.dma_start(wxh[k][:], WxhT[k])
            bxh_t = cpool.tile([128, 8], f32, tag="bxh")
            nc.sync.dma_start(bxh_t[:], bxh[:])
            bg_t = cpool.tile([128, 32], f32, tag="bg")
            nc.sync.dma_start(bg_t[:], bg[:])

            xr_pool = pro.enter_context(tc.tile_pool(name="pxr", bufs=2))
            th_pool = pro.enter_context(tc.tile_pool(name="pth", bufs=2))
            for n in range(NB):
                xr = [xr_pool.tile([128, NT], bf, tag=f"xr{k}", name=f"xr{k}") for k in range(3)]
                for k in range(3):
                    nc.sync.dma_start(xr[k][:], xT[k][:, n * NT : (n + 1) * NT])
                for m in range(8):
                    ps = pspool.tile([128, NT], f32, tag="ps")
                    for k in range(3):
                        nc.tensor.matmul(
                            ps[:],
                            wxh[k][:, m * 128 : (m + 1) * 128],
                            xr[k][:],
                            start=(k == 0),
                            stop=(k == 2),
                        )
                    tht = th_pool.tile([128, NT], bf, tag="tht")
                    nc.scalar.activation(tht[:], ps[:], AF.Tanh, bias=bxh_t[:, m : m + 1])
                    nc.sync.dma_start(th_d[m][:, n * NT : (n + 1) * NT], tht[:])

            # G0 = W_ihA @ t_h + (b_ih + b_hh)   (bf16, pre-tiled by (n, r))
            wa_pool = pro.enter_context(tc.tile_pool(name="pwa", bufs=1))
            wa = [wa_pool.tile([128, G4], bf, tag=f"wa{k}", name=f"wa{k}") for k in range(8)]
            for k in range(8):
                nc.sync.dma_start(wa[k][:], WihAT[k])
            thr_pool = pro.enter_context(tc.tile_pool(name="pthr", bufs=2))
            g0_pool = pro.enter_context(tc.tile_pool(name="pg0", bufs=2))
            for n in range(NB):
                thr = [thr_pool.tile([128, NT], bf, tag=f"thr{k}", name=f"thr{k}") for k in range(8)]
                for k in range(8):
                    nc.sync.dma_start(thr[k][:], th_d[k][:, n * NT : (n + 1) * NT])
                for r in range(8):
                    g0t = g0_pool.tile([128, 4 * NT], bf, tag="g0t")
                    for gi in range(4):
                        m = gi * 8 + r
                        ps = pspool.tile([128, NT], f32, tag="ps")
                        for k in range(8):
                            nc.tensor.matmul(
                                ps[:],
                                wa[k][:, m * 128 : (m + 1) * 128],
                                thr[k][:],
                                start=(k == 0),
                                stop=(k == 7),
                            )
                        nc.scalar.activation(
                            g0t[:, gi * NT : (gi + 1) * NT],
                            ps[:],
                            AF.Identity,
                            bias=bg_t[:, m : m + 1],
                        )
                    nc.sync.dma_start(G0_d[n, r], g0t[:])

        # ================= resident weights =================
        wres = ctx.enter_context(tc.tile_pool(name="wres", bufs=1))
        wh = [wres.tile([128, G4], bf, tag=f"wh{k}", name=f"wh{k}") for k in range(8)]
        for k in range(8):
            nc.sync.dma_start(wh[k][:], WhhT[k])
        wb_t = wres.tile([A, G4], bf, tag="wbig")
        nc.sync.dma_start(wb_t[:], WbigT[:])
        wz = [wres.tile([128, A], bf, tag=f"wz{k}", name=f"wz{k}") for k in range(8)]
        for k in range(8):
            nc.sync.dma_start(wz[k][:], WhzT[k])
        ones_t = wres.tile([A, 1], f32, tag="onesA")
        nc.sync.dma_start(ones_t[:], onesA[:])
        ones1_t = wres.tile([1, 128], f32, tag="ones1")
        nc.sync.dma_start(ones1_t[:], ones1[:])
        bhz_t = wres.tile([A, 1], f32, tag="bhz")
        nc.sync.dma_start(bhz_t[:], bhz[:])
        eps_t = wres.tile([A, 1], f32, tag="eps")
        nc.vector.memset(eps_t[:], EPS)

        # ================= main loop pools =================
        psum = ctx.enter_context(tc.tile_pool(name="psum", bufs=8, space="PSUM"))
        g0r_p = ctx.enter_context(tc.tile_pool(name="g0r", bufs=2))
        cin_p = ctx.enter_context(tc.tile_pool(name="cin", bufs=2))
        cell_p = ctx.enter_context(tc.tile_pool(name="cell", bufs=2))
        hn_p = ctx.enter_context(tc.tile_pool(name="hn", bufs=1))
        zp_p = ctx.enter_context(tc.tile_pool(name="zp", bufs=1))

        def lstm_step(rb, wb, parity, jv):
            """One LSTM step: read state[rb], write state[wb], probs->p_all[parity][jv]."""
            for n in range(NB):
                sl = slice(n * NT, (n + 1) * NT)
                pr = pres[n]
                hr = [hres[k][n] for k in range(8)]
                hnew = []
                for r in range(8):
                    g0t = g0r_p.tile([128, 4 * NT], bf, tag="g0t")
                    nc.sync.dma_start(g0t[:], G0_d[n, r])
                    cin = cin_p.tile([128, NT], f32, tag="cin")
                    nc.sync.dma_start(cin[:], c_d[rb][n, r])
                    gps = []
                    for gi in range(4):
                        m = gi * 8 + r
                        ps = psum.tile([128, NT], f32, tag="ps")
                        for k in range(8):
                            nc.tensor.matmul(
                                ps[:],
                                wh[k][:, m * 128 : (m + 1) * 128],
                                hr[k][:],
                                start=(k == 0),
                                stop=False,
                            )
                        nc.tensor.matmul(
                            ps[:],
                            wb_t[:, m * 128 : (m + 1) * 128],
                            pr[:],
                            start=False,
                            stop=True,
                        )
                        # add G0 and apply gate nonlinearity in-place in PSUM
                        nc.vector.tensor_tensor(
                            ps[:], ps[:], g0t[:, gi * NT : (gi + 1) * NT],
                            mybir.AluOpType.add,
                        )
                        # evict gate activation to SBUF immediately: frees the
                        # PSUM bank after one ACT and keeps the cell math in
                        # SBUF (DVE fast path, no PSUM-read limits)
                        gsb = cell_p.tile(
                            [128, NT], f32, tag=f"gate{gi}", name=f"gate{gi}"
                        )
                        nc.scalar.activation(
                            gsb[:], ps[:], AF.Tanh if gi == 2 else AF.Sigmoid
                        )
                        gps.append(gsb)
                    i_sb, f_sb, g_sb, o_sb = gps
                    # c' = f*c + i*g ; h = o*tanh(c')
                    ig_sb = cell_p.tile([128, NT], f32, tag="igsb")
                    nc.vector.tensor_tensor(ig_sb[:], g_sb[:], i_sb[:], mybir.AluOpType.mult)
                    nc.vector.tensor_tensor(f_sb[:], f_sb[:], cin[:], mybir.AluOpType.mult)
                    cnew = cell_p.tile([128, NT], f32, tag="cnew")
                    nc.vector.tensor_tensor(cnew[:], f_sb[:], ig_sb[:], mybir.AluOpType.add)
                    nc.sync.dma_start(c_d[wb][n, r], cnew[:])
                    tht = cell_p.tile([128, NT], f32, tag="tht")
                    nc.scalar.activation(tht[:], cnew[:], AF.Tanh)
                    # new h goes to a temp first: the resident h[k][n] tiles
                    # are still being read as matmul rhs by later r-groups
                    hbf = hn_p.tile([128, NT], bf, tag=f"hn{r}", name=f"hn{r}")
                    nc.vector.tensor_tensor(hbf[:], o_sb[:], tht[:], mybir.AluOpType.mult)
                    hnew.append(hbf)
                # z/p/e phase
                zps = psum.tile([A, NT], f32, tag="ps")
                for k in range(8):
                    nc.tensor.matmul(
                        zps[:], wz[k][:], hnew[k][:], start=(k == 0), stop=(k == 7)
                    )
                # commit new h into the resident state (all reads of the old
                # value — this chunk's gate matmuls — are earlier in program
                # order, so the WAR is chunk-local and cheap)
                for k in range(8):
                    nc.vector.tensor_copy(hres[k][n][:], hnew[k][:])
                u = zp_p.tile([A, NT], f32, tag="u")
                nc.scalar.activation(u[:], zps[:], AF.Exp, bias=bhz_t[:])
                q2 = zp_p.tile([A, NT], f32, tag="q2")
                nc.scalar.activation(q2[:], u[:], AF.Ln, bias=ones_t[:])
                nc.vector.tensor_scalar_add(q2[:], q2[:], EPS)
                sps = psum.tile([1, NT], f32, tag="ps")
                nc.tensor.matmul(sps[:], ones_t[:], q2[:], start=True, stop=True)
                rec = zp_p.tile([1, NT], f32, tag="rec")
                nc.vector.reciprocal(rec[:], sps[:])
                rbc = psum.tile([128, NT], f32, tag="ps")
                nc.tensor.matmul(rbc[:], ones1_t[:], rec[:], start=True, stop=True)
                pt = zp_p.tile([A, NT], f32, tag="pt")
                nc.vector.tensor_tensor(pt[:], q2[:], rbc[:A, :], mybir.AluOpType.mult)
                if isinstance(jv, int):
                    nc.sync.dma_start(p_all[parity, jv][:, sl], pt[:])
                else:
                    nc.sync.dma_start(p_all[parity][bass.ds(jv, 1)][:, :, sl], pt[:])
                nc.vector.tensor_copy(pres[n][:], pt[:])

        if use_for_i:
            with tc.For_i(0, NS2, 1) as j:
                for u in range(UNROLL):
                    lstm_step(u % 2, (u + 1) % 2, u, j)
        else:
            for t in range(nsteps):
                lstm_step(t % 2, (t + 1) % 2, t % UNROLL, t // UNROLL)

    nc.compile()
    return nc


# ---------------- host-side wrapper ----------------


def _prep_weights(W_xh, b_xh, W_ih, W_hh, b_ih, b_hh, W_hz, b_hz, W_emb):
    bf = ml_dtypes.bfloat16
    f32 = np.float32
    d = {}
    wxh = np.zeros((KXP, H), f32)
    wxh[:E] = np.asarray(W_xh, f32).T
    d["WxhT"] = np.ascontiguousarray(wxh.reshape(3, 128, H)).astype(bf)
    d["bxh"] = np.ascontiguousarray(np.asarray(b_xh, f32).reshape(8, 128).T)
    wih = np.asarray(W_ih, f32)
    d["WihAT"] = np.ascontiguousarray(wih[:, :H].T.reshape(8, 128, G4)).astype(bf)
    wbig = wih[:, H:].astype(np.float64) @ np.asarray(W_emb, np.float64)
    d["WbigT"] = np.ascontiguousarray(wbig.T.astype(np.float32)).astype(bf)
    d["WhhT"] = np.ascontiguousarray(np.asarray(W_hh, f32).T.reshape(8, 128, G4)).astype(bf)
    d["bg"] = np.ascontiguousarray(
        (np.asarray(b_ih, f32) + np.asarray(b_hh, f32)).reshape(32, 128).T
    )
    d["WhzT"] = np.ascontiguousarray(np.asarray(W_hz, f32).T.reshape(8, 128, A)).astype(bf)
    d["bhz"] = np.ascontiguousarray(np.asarray(b_hz, f32).reshape(A, 1))
    d["onesA"] = np.ones((A, 1), f32)
    d["ones1"] = np.ones((1, 128), f32)
    return d


def _prep_x(x_shard):
    bf = ml_dtypes.bfloat16
    xt = np.zeros((KXP, x_shard.shape[0]), np.float32)
    xt[:E] = np.asarray(x_shard, np.float32).T
    return np.ascontiguousarray(xt.reshape(3, 128, -1)).astype(bf)


def kernel(input_x, W_xh, b_xh, W_ih, W_hh, b_ih, b_hh, W_hz, b_hz, W_emb):
    from concourse.bass_utils import run_bass_kernel_spmd

    wd = _prep_weights(W_xh, b_xh, W_ih, W_hh, b_ih, b_hh, W_hz, b_hz, W_emb)
    x = np.asarray(input_x, np.float32)
    in_maps = []
    for c in range(NCORES):
        m = dict(wd)
        m["xT"] = _prep_x(x[c * BL : (c + 1) * BL])
        in_maps.append(m)

    nc = build_nc()
    res = run_bass_kernel_spmd(nc, in_maps, list(range(NCORES)))
    global LAST_RESULT
    LAST_RESULT = res

    out = np.empty((B, D, A), np.float32)
    for c in range(NCORES):
        pa = res.results[c]["p_all"]  # [UNROLL, D//UNROLL, A, BL]
        p = np.empty((D, A, BL), np.float32)
        for u in range(UNROLL):
            p[u::UNROLL] = pa[u]
        out[c * BL : (c + 1) * BL] = p.transpose(2, 0, 1)
    return out, out



# revision 16
# speedup vs baseline: 1.0691x; 1.0691x over previous
"""Trainium2 Bass kernel for AutoRegressiveLSTMEncoder.

Strategy: pure data parallel over 8 NeuronCores (batch 32768 -> 4096/core).
All tensors live feature-on-partition / batch-on-free ("transposed") so every
matmul is lhsT.T @ rhs with K on partitions.

Key algebraic optimizations:
  - softmax(log(softplus(s)+eps)) == (softplus(s)+eps) / sum(softplus(s)+eps)
    -> no exp/log needed, and no max-subtraction (values are bounded).
  - The input-side term W_ih[:, :H] @ t_h + b_ih + b_hh is step-invariant:
    precompute once as G0 (saves 1/3 of the per-step FLOPs).
  - Per-step gates = G0 + W_ih[:, H:] @ e + W_hh @ h  (bf16 matmuls, fp32 acc).

Recurrent state (h as 64 per-(k,chunk) tiles, p per-chunk) lives in SBUF for
all 32 steps -- no DRAM round-trip on the latency-critical recurrence. New h
is computed into temps and committed to the resident tiles only after the
z-phase matmuls consume them (the in-chunk WAR is chunk-local). Only the
step-invariant G0 (read-only) and the cell state c (1-step slack) stream
through DRAM, plus the p_all output.

The 32 LSTM steps run in a For_i hardware loop (8 iterations x 4 steps for
static ping-pong c addressing and fewer loop-boundary syncs); per-step probs
are written phase-major (p_all[4][8][A][B_local]) so the only dynamic address
is the loop counter itself. Host reassembles [B, D, A].
"""

import sys

sys.path.insert(0, "/opt/trn_rl_repo")

import numpy as np
import ml_dtypes
from contextlib import ExitStack

import concourse.bass as bass
import concourse.bacc as bacc
import concourse.tile as tile
from concourse import mybir

AF = mybir.ActivationFunctionType
DT = mybir.dt

# Problem dims (hardcoded per contest contract)
B, E, D, A, H = 32768, 300, 32, 64, 1024
G4 = 4 * H  # 4096
NCORES = 8
BL = B // NCORES  # 4096
NT = 512  # moving free-dim per matmul (one fp32 PSUM bank)
EPS = 1e-6
KXP = 384  # E=300 padded to 3*128


UNROLL = 4


def build_nc(BL=BL, NB=None, nsteps=D, use_for_i=True):
    """Build the SPMD Bass program for one core handling BL batch elements."""
    if NB is None:
        NB = BL // NT
    assert BL == NB * NT and nsteps % UNROLL == 0
    NS2 = nsteps // UNROLL

    nc = bacc.Bacc("TRN2", target_bir_lowering=False, debug=False)
    f32, bf = DT.float32, DT.bfloat16

    # ---- external inputs (host pre-tiled / pre-transposed / pre-cast) ----
    xT = nc.dram_tensor("xT", (3, 128, BL), bf, kind="ExternalInput")
    WxhT = nc.dram_tensor("WxhT", (3, 128, H), bf, kind="ExternalInput")
    bxh = nc.dram_tensor("bxh", (128, 8), f32, kind="ExternalInput")
    WihAT = nc.dram_tensor("WihAT", (8, 128, G4), bf, kind="ExternalInput")
    WbigT = nc.dram_tensor("WbigT", (A, G4), bf, kind="ExternalInput")
    WhhT = nc.dram_tensor("WhhT", (4, 128, 2, G4), DT.float8e4, kind="ExternalInput")
    bg = nc.dram_tensor("bg", (128, 32), f32, kind="ExternalInput")
    WhzT = nc.dram_tensor("WhzT", (8, 128, A), bf, kind="ExternalInput")
    bhz = nc.dram_tensor("bhz", (A, 1), f32, kind="ExternalInput")
    onesA = nc.dram_tensor("onesA", (A, 1), f32, kind="ExternalInput")
    ones1 = nc.dram_tensor("ones1", (1, 128), f32, kind="ExternalInput")

    # ---- output: parity-major probs ----
    p_all = nc.dram_tensor("p_all", (UNROLL, NS2, A, BL), f32, kind="ExternalOutput")

    # ---- internal DRAM scratch ----
    th_d = nc.dram_tensor("th_d", (8, 128, BL), bf, kind="Internal")
    G0_d = nc.dram_tensor("G0_d", (NB, 8, 128, 4 * NT), bf, kind="Internal")
    c_d = [
        nc.dram_tensor(f"c_d{i}", (NB, 8, 128, NT), f32, kind="Internal")
        for i in (0, 1)
    ]

    with tile.TileContext(nc) as tc, ExitStack() as ctx:
        # ---- SBUF-resident recurrent state (h, p stay on-chip all 32 steps;
        # per-(k, n) tiles so WAR tracking is chunk-local) ----
        hres_pool = ctx.enter_context(tc.tile_pool(name="hres", bufs=1))
        f8 = DT.float8e4
        hres = [
            [hres_pool.tile([128, 2, NT], f8, tag=f"h{k}_{n}", name=f"h{k}_{n}") for n in range(NB)]
            for k in range(4)
        ]
        pres = [hres_pool.tile([A, NT], bf, tag=f"p{n}", name=f"p{n}") for n in range(NB)]

        # ================= prologue =================
        with ExitStack() as pro:
            cpool = pro.enter_context(tc.tile_pool(name="pc", bufs=1))
            pspool = pro.enter_context(tc.tile_pool(name="pps", bufs=8, space="PSUM"))

            # zero-init state buffers (set 0)
            ztile = cpool.tile([128, NT], f32, tag="z32")
            nc.vector.memset(ztile[:], 0.0)
            for r in range(8):
                for n in range(NB):
                    nc.sync.dma_start(c_d[0][n, r], ztile[:])
                    if r < 4:
                        nc.vector.memset(hres[r][n][:], 0.0)
            for n in range(NB):
                nc.vector.memset(pres[n][:], 0.0)

            # t_h = tanh(W_xh @ xT + b_xh)
            wxh = [cpool.tile([128, H], bf, tag=f"wxh{k}", name=f"wxh{k}") for k in range(3)]
            for k in range(3):
                nc.sync.dma_start(wxh[k][:], WxhT[k])
            bxh_t = cpool.tile([128, 8], f32, tag="bxh")
            nc.sync.dma_start(bxh_t[:], bxh[:])
            bg_t = cpool.tile([128, 32], f32, tag="bg")
            nc.sync.dma_start(bg_t[:], bg[:])

            xr_pool = pro.enter_context(tc.tile_pool(name="pxr", bufs=2))
            th_pool = pro.enter_context(tc.tile_pool(name="pth", bufs=2))
            for n in range(NB):
                xr = [xr_pool.tile([128, NT], bf, tag=f"xr{k}", name=f"xr{k}") for k in range(3)]
                for k in range(3):
                    nc.sync.dma_start(xr[k][:], xT[k][:, n * NT : (n + 1) * NT])
                for m in range(8):
                    ps = pspool.tile([128, NT], f32, tag="ps")
                    for k in range(3):
                        nc.tensor.matmul(
                            ps[:],
                            wxh[k][:, m * 128 : (m + 1) * 128],
                            xr[k][:],
                            start=(k == 0),
                            stop=(k == 2),
                        )
                    tht = th_pool.tile([128, NT], bf, tag="tht")
                    nc.scalar.activation(tht[:], ps[:], AF.Tanh, bias=bxh_t[:, m : m + 1])
                    nc.sync.dma_start(th_d[m][:, n * NT : (n + 1) * NT], tht[:])

            # G0 = W_ihA @ t_h + (b_ih + b_hh)   (bf16, pre-tiled by (n, r))
            wa_pool = pro.enter_context(tc.tile_pool(name="pwa", bufs=1))
            wa = [wa_pool.tile([128, G4], bf, tag=f"wa{k}", name=f"wa{k}") for k in range(8)]
            for k in range(8):
                nc.sync.dma_start(wa[k][:], WihAT[k])
            thr_pool = pro.enter_context(tc.tile_pool(name="pthr", bufs=2))
            g0_pool = pro.enter_context(tc.tile_pool(name="pg0", bufs=2))
            for n in range(NB):
                thr = [thr_pool.tile([128, NT], bf, tag=f"thr{k}", name=f"thr{k}") for k in range(8)]
                for k in range(8):
                    nc.sync.dma_start(thr[k][:], th_d[k][:, n * NT : (n + 1) * NT])
                for r in range(8):
                    g0t = g0_pool.tile([128, 4 * NT], bf, tag="g0t")
                    for gi in range(4):
                        m = gi * 8 + r
                        ps = pspool.tile([128, NT], f32, tag="ps")
                        for k in range(8):
                            nc.tensor.matmul(
                                ps[:],
                                wa[k][:, m * 128 : (m + 1) * 128],
                                thr[k][:],
                                start=(k == 0),
                                stop=(k == 7),
                            )
                        nc.scalar.activation(
                            g0t[:, gi * NT : (gi + 1) * NT],
                            ps[:],
                            AF.Identity,
                            bias=bg_t[:, m : m + 1],
                        )
                    nc.sync.dma_start(G0_d[n, r], g0t[:])

        # ================= resident weights =================
        wres = ctx.enter_context(tc.tile_pool(name="wres", bufs=1))
        wh = [wres.tile([128, 2, G4], DT.float8e4, tag=f"wh{k}", name=f"wh{k}") for k in range(4)]
        for k in range(4):
            nc.sync.dma_start(wh[k][:], WhhT[k])
        wb_t = wres.tile([A, G4], bf, tag="wbig")
        nc.sync.dma_start(wb_t[:], WbigT[:])
        wz = [wres.tile([128, A], bf, tag=f"wz{k}", name=f"wz{k}") for k in range(8)]
        for k in range(8):
            nc.sync.dma_start(wz[k][:], WhzT[k])
        ones_t = wres.tile([A, 1], f32, tag="onesA")
        nc.sync.dma_start(ones_t[:], onesA[:])
        ones1_t = wres.tile([1, 128], f32, tag="ones1")
        nc.sync.dma_start(ones1_t[:], ones1[:])
        bhz_t = wres.tile([A, 1], f32, tag="bhz")
        nc.sync.dma_start(bhz_t[:], bhz[:])
        eps_t = wres.tile([A, 1], f32, tag="eps")
        nc.vector.memset(eps_t[:], EPS)

        # ================= main loop pools =================
        psum = ctx.enter_context(tc.tile_pool(name="psum", bufs=8, space="PSUM"))
        g0r_p = ctx.enter_context(tc.tile_pool(name="g0r", bufs=2))
        cin_p = ctx.enter_context(tc.tile_pool(name="cin", bufs=2))
        cell_p = ctx.enter_context(tc.tile_pool(name="cell", bufs=2))
        hn_p = ctx.enter_context(tc.tile_pool(name="hn", bufs=1))
        zp_p = ctx.enter_context(tc.tile_pool(name="zp", bufs=1))

        def lstm_step(rb, wb, parity, jv):
            """One LSTM step: read state[rb], write state[wb], probs->p_all[parity][jv]."""
            for n in range(NB):
                sl = slice(n * NT, (n + 1) * NT)
                pr = pres[n]
                hnew = []
                for r in range(8):
                    g0t = g0r_p.tile([128, 4 * NT], bf, tag="g0t")
                    nc.sync.dma_start(g0t[:], G0_d[n, r])
                    cin = cin_p.tile([128, NT], f32, tag="cin")
                    nc.sync.dma_start(cin[:], c_d[rb][n, r])
                    gps = []
                    for gi in range(4):
                        m = gi * 8 + r
                        ps = psum.tile([128, NT], f32, tag="ps")
                        for k in range(4):
                            nc.tensor.matmul(
                                ps[:],
                                wh[k][:, :, m * 128 : (m + 1) * 128],
                                hres[k][n][:],
                                start=(k == 0),
                                stop=False,
                                perf_mode=mybir.MatmulPerfMode.DoubleRow,
                            )
                        nc.tensor.matmul(
                            ps[:],
                            wb_t[:, m * 128 : (m + 1) * 128],
                            pr[:],
                            start=False,
                            stop=True,
                        )
                        # add G0 and apply gate nonlinearity in-place in PSUM
                        nc.vector.tensor_tensor(
                            ps[:], ps[:], g0t[:, gi * NT : (gi + 1) * NT],
                            mybir.AluOpType.add,
                        )
                        # evict gate activation to SBUF immediately: frees the
                        # PSUM bank after one ACT and keeps the cell math in
                        # SBUF (DVE fast path, no PSUM-read limits)
                        gsb = cell_p.tile(
                            [128, NT], f32, tag=f"gate{gi}", name=f"gate{gi}"
                        )
                        nc.scalar.activation(
                            gsb[:], ps[:], AF.Tanh if gi == 2 else AF.Sigmoid
                        )
                        gps.append(gsb)
                    i_sb, f_sb, g_sb, o_sb = gps
                    # c' = f*c + i*g ; h = o*tanh(c')
                    ig_sb = cell_p.tile([128, NT], f32, tag="igsb")
                    nc.vector.tensor_tensor(ig_sb[:], g_sb[:], i_sb[:], mybir.AluOpType.mult)
                    nc.vector.tensor_tensor(f_sb[:], f_sb[:], cin[:], mybir.AluOpType.mult)
                    cnew = cell_p.tile([128, NT], f32, tag="cnew")
                    nc.vector.tensor_tensor(cnew[:], f_sb[:], ig_sb[:], mybir.AluOpType.add)
                    nc.sync.dma_start(c_d[wb][n, r], cnew[:])
                    tht = cell_p.tile([128, NT], f32, tag="tht")
                    nc.scalar.activation(tht[:], cnew[:], AF.Tanh)
                    # new h goes to a temp first: the resident h[k][n] tiles
                    # are still being read as matmul rhs by later r-groups
                    hbf = hn_p.tile([128, NT], bf, tag=f"hn{r}", name=f"hn{r}")
                    nc.vector.tensor_tensor(hbf[:], o_sb[:], tht[:], mybir.AluOpType.mult)
                    hnew.append(hbf)
                # z/p/e phase
                zps = psum.tile([A, NT], f32, tag="ps")
                for k in range(8):
                    nc.tensor.matmul(
                        zps[:], wz[k][:], hnew[k][:], start=(k == 0), stop=(k == 7)
                    )
                # commit new h into the resident state (all reads of the old
                # value — this chunk's gate matmuls — are earlier in program
                # order, so the WAR is chunk-local and cheap)
                for k in range(8):
                    nc.vector.tensor_copy(hres[k // 2][n][:, k % 2], hnew[k][:])
                u = zp_p.tile([A, NT], f32, tag="u")
                nc.scalar.activation(u[:], zps[:], AF.Exp, bias=bhz_t[:])
                q2 = zp_p.tile([A, NT], f32, tag="q2")
                nc.scalar.activation(q2[:], u[:], AF.Ln, bias=ones_t[:])
                nc.vector.tensor_scalar_add(q2[:], q2[:], EPS)
                sps = psum.tile([1, NT], f32, tag="ps")
                nc.tensor.matmul(sps[:], ones_t[:], q2[:], start=True, stop=True)
                rec = zp_p.tile([1, NT], f32, tag="rec")
                nc.vector.reciprocal(rec[:], sps[:])
                rbc = psum.tile([128, NT], f32, tag="ps")
                nc.tensor.matmul(rbc[:], ones1_t[:], rec[:], start=True, stop=True)
                pt = zp_p.tile([A, NT], f32, tag="pt")
                nc.vector.tensor_tensor(pt[:], q2[:], rbc[:A, :], mybir.AluOpType.mult)
                if isinstance(jv, int):
                    nc.sync.dma_start(p_all[parity, jv][:, sl], pt[:])
                else:
                    nc.sync.dma_start(p_all[parity][bass.ds(jv, 1)][:, :, sl], pt[:])
                nc.vector.tensor_copy(pres[n][:], pt[:])

        if use_for_i:
            with tc.For_i(0, NS2, 1) as j:
                for u in range(UNROLL):
                    lstm_step(u % 2, (u + 1) % 2, u, j)
        else:
            for t in range(nsteps):
                lstm_step(t % 2, (t + 1) % 2, t % UNROLL, t // UNROLL)

    nc.compile()
    return nc


# ---------------- host-side wrapper ----------------


def _prep_weights(W_xh, b_xh, W_ih, W_hh, b_ih, b_hh, W_hz, b_hz, W_emb):
    bf = ml_dtypes.bfloat16
    f32 = np.float32
    d = {}
    wxh = np.zeros((KXP, H), f32)
    wxh[:E] = np.asarray(W_xh, f32).T
    d["WxhT"] = np.ascontiguousarray(wxh.reshape(3, 128, H)).astype(bf)
    d["bxh"] = np.ascontiguousarray(np.asarray(b_xh, f32).reshape(8, 128).T)
    wih = np.asarray(W_ih, f32)
    d["WihAT"] = np.ascontiguousarray(wih[:, :H].T.reshape(8, 128, G4)).astype(bf)
    wbig = wih[:, H:].astype(np.float64) @ np.asarray(W_emb, np.float64)
    d["WbigT"] = np.ascontiguousarray(wbig.T.astype(np.float32)).astype(bf)
    f8 = ml_dtypes.float8_e4m3
    whh = np.asarray(W_hh, f32).T.reshape(4, 2, 128, G4).transpose(0, 2, 1, 3)
    d["WhhT"] = np.ascontiguousarray(whh).astype(f8)
    d["bg"] = np.ascontiguousarray(
        (np.asarray(b_ih, f32) + np.asarray(b_hh, f32)).reshape(32, 128).T
    )
    d["WhzT"] = np.ascontiguousarray(np.asarray(W_hz, f32).T.reshape(8, 128, A)).astype(bf)
    d["bhz"] = np.ascontiguousarray(np.asarray(b_hz, f32).reshape(A, 1))
    d["onesA"] = np.ones((A, 1), f32)
    d["ones1"] = np.ones((1, 128), f32)
    return d


def _prep_x(x_shard):
    bf = ml_dtypes.bfloat16
    xt = np.zeros((KXP, x_shard.shape[0]), np.float32)
    xt[:E] = np.asarray(x_shard, np.float32).T
    return np.ascontiguousarray(xt.reshape(3, 128, -1)).astype(bf)


def kernel(input_x, W_xh, b_xh, W_ih, W_hh, b_ih, b_hh, W_hz, b_hz, W_emb):
    from concourse.bass_utils import run_bass_kernel_spmd

    wd = _prep_weights(W_xh, b_xh, W_ih, W_hh, b_ih, b_hh, W_hz, b_hz, W_emb)
    x = np.asarray(input_x, np.float32)
    in_maps = []
    for c in range(NCORES):
        m = dict(wd)
        m["xT"] = _prep_x(x[c * BL : (c + 1) * BL])
        in_maps.append(m)

    nc = build_nc()
    res = run_bass_kernel_spmd(nc, in_maps, list(range(NCORES)))
    global LAST_RESULT
    LAST_RESULT = res

    out = np.empty((B, D, A), np.float32)
    for c in range(NCORES):
        pa = res.results[c]["p_all"]  # [UNROLL, D//UNROLL, A, BL]
        p = np.empty((D, A, BL), np.float32)
        for u in range(UNROLL):
            p[u::UNROLL] = pa[u]
        out[c * BL : (c + 1) * BL] = p.transpose(2, 0, 1)
    return out, out



# revision 17
# speedup vs baseline: 1.2404x; 1.1602x over previous
"""Trainium2 Bass kernel for AutoRegressiveLSTMEncoder.

Strategy: pure data parallel over 8 NeuronCores (batch 32768 -> 4096/core).
All tensors live feature-on-partition / batch-on-free ("transposed") so every
matmul is lhsT.T @ rhs with K on partitions.

Key algebraic optimizations:
  - softmax(log(softplus(s)+eps)) == (softplus(s)+eps) / sum(softplus(s)+eps)
    -> no exp/log needed, and no max-subtraction (values are bounded).
  - The input-side term W_ih[:, :H] @ t_h + b_ih + b_hh is step-invariant:
    precompute once as G0 (saves 1/3 of the per-step FLOPs).
  - Per-step gates = G0 + W_ih[:, H:] @ e + W_hh @ h  (bf16 matmuls, fp32 acc).

Recurrent state (h as 64 per-(k,chunk) tiles, p per-chunk) lives in SBUF for
all 32 steps -- no DRAM round-trip on the latency-critical recurrence. New h
is computed into temps and committed to the resident tiles only after the
z-phase matmuls consume them (the in-chunk WAR is chunk-local). Only the
step-invariant G0 (read-only) and the cell state c (1-step slack) stream
through DRAM, plus the p_all output.

The 32 LSTM steps run in a For_i hardware loop (8 iterations x 4 steps for
static ping-pong c addressing and fewer loop-boundary syncs); per-step probs
are written phase-major (p_all[4][8][A][B_local]) so the only dynamic address
is the loop counter itself. Host reassembles [B, D, A].
"""

import sys

sys.path.insert(0, "/opt/trn_rl_repo")

import numpy as np
import ml_dtypes
from contextlib import ExitStack

import concourse.bass as bass
import concourse.bacc as bacc
import concourse.tile as tile
from concourse import mybir

AF = mybir.ActivationFunctionType
DT = mybir.dt

# Problem dims (hardcoded per contest contract)
B, E, D, A, H = 32768, 300, 32, 64, 1024
G4 = 4 * H  # 4096
NCORES = 8
BL = B // NCORES  # 4096
NT = 512  # moving free-dim per matmul (one fp32 PSUM bank)
EPS = 1e-6
KXP = 384  # E=300 padded to 3*128


UNROLL = 4


def build_nc(BL=BL, NB=None, nsteps=D, use_for_i=True):
    """Build the SPMD Bass program for one core handling BL batch elements."""
    if NB is None:
        NB = BL // NT
    assert BL == NB * NT and nsteps % UNROLL == 0
    NS2 = nsteps // UNROLL

    nc = bacc.Bacc("TRN2", target_bir_lowering=False, debug=False)
    f32, bf = DT.float32, DT.bfloat16

    # ---- external inputs (host pre-tiled / pre-transposed / pre-cast) ----
    xT = nc.dram_tensor("xT", (3, 128, BL), bf, kind="ExternalInput")
    WxhT = nc.dram_tensor("WxhT", (3, 128, H), bf, kind="ExternalInput")
    bxh = nc.dram_tensor("bxh", (128, 8), f32, kind="ExternalInput")
    WihAT = nc.dram_tensor("WihAT", (8, 128, G4), bf, kind="ExternalInput")
    WbigT = nc.dram_tensor("WbigT", (A, G4), bf, kind="ExternalInput")
    WhhT = nc.dram_tensor("WhhT", (8, 128, G4), bf, kind="ExternalInput")
    bg = nc.dram_tensor("bg", (128, 32), f32, kind="ExternalInput")
    WhzT = nc.dram_tensor("WhzT", (8, 128, A), bf, kind="ExternalInput")
    bhz = nc.dram_tensor("bhz", (A, 1), f32, kind="ExternalInput")
    onesA = nc.dram_tensor("onesA", (A, 1), f32, kind="ExternalInput")
    ones1 = nc.dram_tensor("ones1", (1, 128), f32, kind="ExternalInput")

    # ---- output: parity-major probs ----
    p_all = nc.dram_tensor("p_all", (UNROLL, NS2, A, BL), f32, kind="ExternalOutput")

    # ---- internal DRAM scratch ----
    th_d = nc.dram_tensor("th_d", (8, 128, BL), bf, kind="Internal")
    G0_d = nc.dram_tensor("G0_d", (NB, 8, 128, 4 * NT), bf, kind="Internal")
    c_d = [
        nc.dram_tensor(f"c_d{i}", (NB, 8, 128, NT), f32, kind="Internal")
        for i in (0, 1)
    ]

    with tile.TileContext(nc) as tc, ExitStack() as ctx:
        # ---- SBUF-resident recurrent state (h, p stay on-chip all 32 steps;
        # per-(k, n) tiles so WAR tracking is chunk-local) ----
        hres_pool = ctx.enter_context(tc.tile_pool(name="hres", bufs=1))
        hres = [
            [hres_pool.tile([128, NT], bf, tag=f"h{k}_{n}", name=f"h{k}_{n}") for n in range(NB)]
            for k in range(8)
        ]
        pres = [hres_pool.tile([A, NT], bf, tag=f"p{n}", name=f"p{n}") for n in range(NB)]

        # ================= prologue =================
        with ExitStack() as pro:
            cpool = pro.enter_context(tc.tile_pool(name="pc", bufs=1))
            pspool = pro.enter_context(tc.tile_pool(name="pps", bufs=8, space="PSUM"))

            # zero-init state buffers (set 0)
            ztile = cpool.tile([128, NT], f32, tag="z32")
            nc.vector.memset(ztile[:], 0.0)
            for r in range(8):
                for n in range(NB):
                    nc.sync.dma_start(c_d[0][n, r], ztile[:])
                    nc.vector.memset(hres[r][n][:], 0.0)
            for n in range(NB):
                nc.vector.memset(pres[n][:], 0.0)

            # t_h = tanh(W_xh @ xT + b_xh)
            wxh = [cpool.tile([128, H], bf, tag=f"wxh{k}", name=f"wxh{k}") for k in range(3)]
            for k in range(3):
                nc.sync.dma_start(wxh[k][:], WxhT[k])
            bxh_t = cpool.tile([128, 8], f32, tag="bxh")
            nc.sync.dma_start(bxh_t[:], bxh[:])
            bg_t = cpool.tile([128, 32], f32, tag="bg")
            nc.sync.dma_start(bg_t[:], bg[:])

            xr_pool = pro.enter_context(tc.tile_pool(name="pxr", bufs=2))
            th_pool = pro.enter_context(tc.tile_pool(name="pth", bufs=2))
            for n in range(NB):
                xr = [xr_pool.tile([128, NT], bf, tag=f"xr{k}", name=f"xr{k}") for k in range(3)]
                for k in range(3):
                    nc.sync.dma_start(xr[k][:], xT[k][:, n * NT : (n + 1) * NT])
                for m in range(8):
                    ps = pspool.tile([128, NT], f32, tag="ps")
                    for k in range(3):
                        nc.tensor.matmul(
                            ps[:],
                            wxh[k][:, m * 128 : (m + 1) * 128],
                            xr[k][:],
                            start=(k == 0),
                            stop=(k == 2),
                        )
                    tht = th_pool.tile([128, NT], bf, tag="tht")
                    nc.scalar.activation(tht[:], ps[:], AF.Tanh, bias=bxh_t[:, m : m + 1])
                    nc.sync.dma_start(th_d[m][:, n * NT : (n + 1) * NT], tht[:])

            # G0 = W_ihA @ t_h + (b_ih + b_hh)   (bf16, pre-tiled by (n, r))
            wa_pool = pro.enter_context(tc.tile_pool(name="pwa", bufs=1))
            wa = [wa_pool.tile([128, G4], bf, tag=f"wa{k}", name=f"wa{k}") for k in range(8)]
            for k in range(8):
                nc.sync.dma_start(wa[k][:], WihAT[k])
            thr_pool = pro.enter_context(tc.tile_pool(name="pthr", bufs=2))
            g0_pool = pro.enter_context(tc.tile_pool(name="pg0", bufs=2))
            for n in range(NB):
                thr = [thr_pool.tile([128, NT], bf, tag=f"thr{k}", name=f"thr{k}") for k in range(8)]
                for k in range(8):
                    nc.sync.dma_start(thr[k][:], th_d[k][:, n * NT : (n + 1) * NT])
                for r in range(8):
                    g0t = g0_pool.tile([128, 4 * NT], bf, tag="g0t")
                    for gi in range(4):
                        m = gi * 8 + r
                        ps = pspool.tile([128, NT], f32, tag="ps")
                        for k in range(8):
                            nc.tensor.matmul(
                                ps[:],
                                wa[k][:, m * 128 : (m + 1) * 128],
                                thr[k][:],
                                start=(k == 0),
                                stop=(k == 7),
                            )
                        nc.scalar.activation(
                            g0t[:, gi * NT : (gi + 1) * NT],
                            ps[:],
                            AF.Identity,
                            bias=bg_t[:, m : m + 1],
                        )
                    nc.sync.dma_start(G0_d[n, r], g0t[:])

        # ================= resident weights =================
        wres = ctx.enter_context(tc.tile_pool(name="wres", bufs=1))
        wh = [wres.tile([128, G4], bf, tag=f"wh{k}", name=f"wh{k}") for k in range(8)]
        for k in range(8):
            nc.sync.dma_start(wh[k][:], WhhT[k])
        wb_t = wres.tile([A, G4], bf, tag="wbig")
        nc.sync.dma_start(wb_t[:], WbigT[:])
        wz = [wres.tile([128, A], bf, tag=f"wz{k}", name=f"wz{k}") for k in range(8)]
        for k in range(8):
            nc.sync.dma_start(wz[k][:], WhzT[k])
        ones_t = wres.tile([A, 1], f32, tag="onesA")
        nc.sync.dma_start(ones_t[:], onesA[:])
        ones1_t = wres.tile([1, 128], f32, tag="ones1")
        nc.sync.dma_start(ones1_t[:], ones1[:])
        bhz_t = wres.tile([A, 1], f32, tag="bhz")
        nc.sync.dma_start(bhz_t[:], bhz[:])
        eps_t = wres.tile([A, 1], f32, tag="eps")
        nc.vector.memset(eps_t[:], EPS)

        # ================= main loop pools =================
        psum = ctx.enter_context(tc.tile_pool(name="psum", bufs=8, space="PSUM"))
        g0r_p = ctx.enter_context(tc.tile_pool(name="g0r", bufs=2))
        cin_p = ctx.enter_context(tc.tile_pool(name="cin", bufs=2))
        cell_p = ctx.enter_context(tc.tile_pool(name="cell", bufs=2))
        hn_p = ctx.enter_context(tc.tile_pool(name="hn", bufs=1))
        zp_p = ctx.enter_context(tc.tile_pool(name="zp", bufs=1))

        def lstm_step(rb, wb, parity, jv):
            """One LSTM step: read state[rb], write state[wb], probs->p_all[parity][jv]."""
            for n in range(NB):
                sl = slice(n * NT, (n + 1) * NT)
                pr = pres[n]
                hr = [hres[k][n] for k in range(8)]
                hnew = []
                for r in range(8):
                    g0t = g0r_p.tile([128, 4 * NT], bf, tag="g0t")
                    nc.sync.dma_start(g0t[:], G0_d[n, r])
                    cin = cin_p.tile([128, NT], f32, tag="cin")
                    nc.sync.dma_start(cin[:], c_d[rb][n, r])
                    gps = []
                    for gi in range(4):
                        m = gi * 8 + r
                        ps = psum.tile([128, NT], f32, tag="ps")
                        for k in range(8):
                            nc.tensor.matmul(
                                ps[:],
                                wh[k][:, m * 128 : (m + 1) * 128],
                                hr[k][:],
                                start=(k == 0),
                                stop=False,
                            )
                        nc.tensor.matmul(
                            ps[:],
                            wb_t[:, m * 128 : (m + 1) * 128],
                            pr[:],
                            start=False,
                            stop=True,
                        )
                        # add G0 and apply gate nonlinearity in-place in PSUM
                        nc.vector.tensor_tensor(
                            ps[:], ps[:], g0t[:, gi * NT : (gi + 1) * NT],
                            mybir.AluOpType.add,
                        )
                        # evict gate activation to SBUF immediately: frees the
                        # PSUM bank after one ACT and keeps the cell math in
                        # SBUF (DVE fast path, no PSUM-read limits)
                        gsb = cell_p.tile(
                            [128, NT], f32, tag=f"gate{gi}", name=f"gate{gi}"
                        )
                        nc.scalar.activation(
                            gsb[:], ps[:], AF.Tanh if gi == 2 else AF.Sigmoid
                        )
                        gps.append(gsb)
                    i_sb, f_sb, g_sb, o_sb = gps
                    # c' = f*c + i*g ; h = o*tanh(c')
                    ig_sb = cell_p.tile([128, NT], f32, tag="igsb")
                    nc.vector.tensor_tensor(ig_sb[:], g_sb[:], i_sb[:], mybir.AluOpType.mult)
                    nc.vector.tensor_tensor(f_sb[:], f_sb[:], cin[:], mybir.AluOpType.mult)
                    cnew = cell_p.tile([128, NT], f32, tag="cnew")
                    nc.vector.tensor_tensor(cnew[:], f_sb[:], ig_sb[:], mybir.AluOpType.add)
                    nc.sync.dma_start(c_d[wb][n, r], cnew[:])
                    tht = cell_p.tile([128, NT], f32, tag="tht")
                    nc.scalar.activation(tht[:], cnew[:], AF.Tanh)
                    # new h goes to a temp first: the resident h[k][n] tiles
                    # are still being read as matmul rhs by later r-groups
                    hbf = hn_p.tile([128, NT], bf, tag=f"hn{r}", name=f"hn{r}")
                    nc.vector.tensor_tensor(hbf[:], o_sb[:], tht[:], mybir.AluOpType.mult)
                    hnew.append(hbf)
                # z/p/e phase
                zps = psum.tile([A, NT], f32, tag="ps")
                for k in range(8):
                    nc.tensor.matmul(
                        zps[:], wz[k][:], hnew[k][:], start=(k == 0), stop=(k == 7)
                    )
                # commit new h into the resident state (all reads of the old
                # value — this chunk's gate matmuls — are earlier in program
                # order, so the WAR is chunk-local and cheap)
                for k in range(8):
                    nc.vector.tensor_copy(hres[k][n][:], hnew[k][:])
                u = zp_p.tile([A, NT], f32, tag="u")
                nc.scalar.activation(u[:], zps[:], AF.Exp, bias=bhz_t[:])
                q2 = zp_p.tile([A, NT], f32, tag="q2")
                nc.scalar.activation(q2[:], u[:], AF.Ln, bias=ones_t[:])
                nc.vector.tensor_scalar_add(q2[:], q2[:], EPS)
                sps = psum.tile([1, NT], f32, tag="ps")
                nc.tensor.matmul(sps[:], ones_t[:], q2[:], start=True, stop=True)
                rec = zp_p.tile([1, NT], f32, tag="rec")
                nc.vector.reciprocal(rec[:], sps[:])
                rbc = psum.tile([128, NT], f32, tag="ps")
                nc.tensor.matmul(rbc[:], ones1_t[:], rec[:], start=True, stop=True)
                pt = zp_p.tile([A, NT], f32, tag="pt")
                nc.vector.tensor_tensor(pt[:], q2[:], rbc[:A, :], mybir.AluOpType.mult)
                if isinstance(jv, int):
                    nc.sync.dma_start(p_all[parity, jv][:, sl], pt[:])
                else:
                    nc.sync.dma_start(p_all[parity][bass.ds(jv, 1)][:, :, sl], pt[:])
                nc.vector.tensor_copy(pres[n][:], pt[:])

        if use_for_i:
            with tc.For_i(0, NS2, 1) as j:
                for u in range(UNROLL):
                    lstm_step(u % 2, (u + 1) % 2, u, j)
        else:
            for t in range(nsteps):
                lstm_step(t % 2, (t + 1) % 2, t % UNROLL, t // UNROLL)

    nc.compile()
    return nc


# ---------------- host-side wrapper ----------------


def _prep_weights(W_xh, b_xh, W_ih, W_hh, b_ih, b_hh, W_hz, b_hz, W_emb):
    bf = ml_dtypes.bfloat16
    f32 = np.float32
    d = {}
    wxh = np.zeros((KXP, H), f32)
    wxh[:E] = np.asarray(W_xh, f32).T
    d["WxhT"] = np.ascontiguousarray(wxh.reshape(3, 128, H)).astype(bf)
    d["bxh"] = np.ascontiguousarray(np.asarray(b_xh, f32).reshape(8, 128).T)
    wih = np.asarray(W_ih, f32)
    d["WihAT"] = np.ascontiguousarray(wih[:, :H].T.reshape(8, 128, G4)).astype(bf)
    wbig = wih[:, H:].astype(np.float64) @ np.asarray(W_emb, np.float64)
    d["WbigT"] = np.ascontiguousarray(wbig.T.astype(np.float32)).astype(bf)
    d["WhhT"] = np.ascontiguousarray(np.asarray(W_hh, f32).T.reshape(8, 128, G4)).astype(bf)
    d["bg"] = np.ascontiguousarray(
        (np.asarray(b_ih, f32) + np.asarray(b_hh, f32)).reshape(32, 128).T
    )
    d["WhzT"] = np.ascontiguousarray(np.asarray(W_hz, f32).T.reshape(8, 128, A)).astype(bf)
    d["bhz"] = np.ascontiguousarray(np.asarray(b_hz, f32).reshape(A, 1))
    d["onesA"] = np.ones((A, 1), f32)
    d["ones1"] = np.ones((1, 128), f32)
    return d


def _prep_x(x_shard):
    bf = ml_dtypes.bfloat16
    xt = np.zeros((KXP, x_shard.shape[0]), np.float32)
    xt[:E] = np.asarray(x_shard, np.float32).T
    return np.ascontiguousarray(xt.reshape(3, 128, -1)).astype(bf)


def kernel(input_x, W_xh, b_xh, W_ih, W_hh, b_ih, b_hh, W_hz, b_hz, W_emb):
    from concourse.bass_utils import run_bass_kernel_spmd

    wd = _prep_weights(W_xh, b_xh, W_ih, W_hh, b_ih, b_hh, W_hz, b_hz, W_emb)
    x = np.asarray(input_x, np.float32)
    in_maps = []
    for c in range(NCORES):
        m = dict(wd)
        m["xT"] = _prep_x(x[c * BL : (c + 1) * BL])
        in_maps.append(m)

    nc = build_nc()
    res = run_bass_kernel_spmd(nc, in_maps, list(range(NCORES)))
    global LAST_RESULT
    LAST_RESULT = res

    out = np.empty((B, D, A), np.float32)
    for c in range(NCORES):
        pa = res.results[c]["p_all"]  # [UNROLL, D//UNROLL, A, BL]
        p = np.empty((D, A, BL), np.float32)
        for u in range(UNROLL):
            p[u::UNROLL] = pa[u]
        out[c * BL : (c + 1) * BL] = p.transpose(2, 0, 1)
    return out, out

